# revision 40
# baseline (speedup 1.0000x reference)
"""
Trainium2 distributed kernel for causal multi-head attention
(nn_Attention: B=4, S=2048, D=768, H=4, DH=192).

Sharding: 16 (batch, head) units across 8 cores = 1 batch x 2 heads per
core.  Every core runs an identical graph (SPMD) on its own shard; the
host sums core pairs (the unshard for output-partial sharding).  No
on-device collectives, perfectly balanced causal work.

Device algorithm (bf16 matmuls, f32 PSUM accumulation):
  QT/KT stored transposed [head-dim planes, seq]; the two heads' upper
  64 head-dims share one 128-partition plane (host permutes weight
  columns to match), so every projection matmul contracts a full 128
  partitions.  Scores are computed transposed, S.T[k, q] = KT.T @ QT;
  causal diagonal blocks trim the moving (q) range to the live columns.
  Softmax skips max-subtraction (logits are O(1) by construction);
  causality is applied post-exp as a multiplicative 0/1 bf16 mask on
  the diagonal blocks only.  AV runs attention-stationary: for each
  128-query sub-block the exp'd score block is the stationary operand
  and V (with an appended ones column) is the moving operand, so the
  PE runs at full width (193 useful columns) and the softmax
  denominator lands as a per-partition column - normalization is one
  DVE reciprocal + one tensor_scalar multiply, no broadcast matmuls.
  The normalized [q, d] tile is transposed back to [d, q] planes by the
  DMA xbar (async, off every engine), feeding the deferred output
  projection unchanged.  The PE is pre-warmed with dummy matmuls during
  the DMA lead-in so real matmuls start at full clock.
"""

import math
import os
import sys

import numpy as np

for _p in ("/opt/trn_rl_repo",):
    if _p not in sys.path and os.path.isdir(_p):
        sys.path.insert(0, _p)

import ml_dtypes  # noqa: E402

B, S, D, H = 4, 2048, 768, 4
DH = D // H  # 192
HPC = 2  # heads per core
HD = HPC * DH  # 384 head dims per core
P = 128
KD = D // P  # 6 contraction chunks over D
QB = 512  # query block (matmul moving dim)
NQ = S // QB  # 4
KB = 128  # key block (psum partition dim)
NK = S // KB  # 16
MS = S // P  # 16 seq chunks
SCALE = 1.0 / math.sqrt(DH)

# host-side column permutation for Wq/Wk:
# planes = [h0 dh0:128 | h1 dh0:128 | h0 dh128:192, h1 dh128:192]
PQ = np.r_[0:128, 192:320, 128:192, 320:384]

_CACHED = {}


def build_nc(reps=1):
    import concourse.mybir as mybir
    from concourse import bacc
    from concourse import tile

    fp32 = mybir.dt.float32
    bf16 = mybir.dt.bfloat16
    Exp = mybir.ActivationFunctionType.Exp

    nc = bacc.Bacc(None, target_bir_lowering=False)

    xT = nc.declare_dram_parameter("xT", [D, S], bf16, isOutput=False)
    wqT = nc.declare_dram_parameter("wqT", [D, HD], bf16, isOutput=False)
    wkT = nc.declare_dram_parameter("wkT", [D, HD], bf16, isOutput=False)
    wvT = nc.declare_dram_parameter("wvT", [D, HD], bf16, isOutput=False)
    woS = nc.declare_dram_parameter("woS", [HD, D], bf16, isOutput=False)
    out = nc.declare_dram_parameter("out", [S, D], bf16, isOutput=True)

    # V sbuf free layout per key block: [h0 dh(192), ones | h1 dh(192), ones]
    VW = 386

    with tile.TileContext(nc) as tc:
        with (
            tc.tile_pool(name="const", bufs=1) as const,
            tc.tile_pool(name="atp", bufs=2) as atp,
            tc.tile_pool(name="ost", bufs=4) as ostp,
            tc.tile_pool(name="natp", bufs=4) as natp,
            tc.tile_pool(name="rcp", bufs=4) as rcp,
            tc.tile_pool(name="scps", bufs=2, space="PSUM") as scps,
            tc.tile_pool(name="avps", bufs=1, space="PSUM") as avps,
        ):
            # ---- persistent SBUF tensors ----
            xT_sb = const.tile([P, KD, S], bf16, tag="xT_sb")
            wq_sb = const.tile([P, KD, HD], bf16, tag="wq_sb")
            wk_sb = const.tile([P, KD, HD], bf16, tag="wk_sb")
            wv_sb = const.tile([P, KD, HD], bf16, tag="wv_sb")
            wo_sb = const.tile([P, 3, D], bf16, tag="wo_sb")
            qt_sb = const.tile([P, 3, S], bf16, tag="qt_sb")
            kt_sb = const.tile([P, 3, S], bf16, tag="kt_sb")
            v_sb = const.tile([P, NK, VW], bf16, tag="v_sb")
            pt_sb = const.tile([P, 3, S], bf16, tag="pt_sb")
            ones1 = const.tile([1, P], bf16, tag="ones1")
            warm = const.tile([1, 1], fp32, tag="warm")
            wtile = const.tile([P, QB], bf16, tag="wtile")
            masks = const.tile([P, 4, 2 * QB], bf16, tag="masks")

            # prefetch the exp table + pre-warm the PE clock gate with
            # dummy matmuls while the input DMAs stream in
            nc.vector.memset(ones1[:], 1.0)
            nc.scalar.activation(warm[:], ones1[0:1, 0:1], Exp)
            nc.vector.memset(wtile[:], 0.0)
            for i in range(26):
                wps = scps.tile([P, QB], fp32, tag="sc", name=f"warm{i}")
                nc.tensor.matmul(
                    wps, lhsT=wtile[:, 0:P], rhs=wtile[:], start=True, stop=True
                )

            # ---- input DMAs ----
            # weights and x planes interleaved on the two HWDGE queues so
            # each plane's (wq_k, wk_k, x_k) trio lands together; the first
            # x plane is chunked so the first matmuls fire early
            nc.sync.dma_start(wq_sb[:, 0, :], wqT[0:P, :])
            nc.scalar.dma_start(wk_sb[:, 0, :], wkT[0:P, :])
            nc.scalar.dma_start(wq_sb[:, 1, :], wqT[P : 2 * P, :])
            nc.sync.dma_start(wk_sb[:, 1, :], wkT[P : 2 * P, :])
            for c in range(4):
                nc.sync.dma_start(
                    xT_sb[:, 0, c * QB : (c + 1) * QB],
                    xT[0:P, c * QB : (c + 1) * QB],
                )
            for c in range(2):
                nc.scalar.dma_start(
                    xT_sb[:, 1, c * 1024 : (c + 1) * 1024],
                    xT[P : 2 * P, c * 1024 : (c + 1) * 1024],
                )
            for k in range(2, KD):
                qA, qB = (nc.sync, nc.scalar) if k % 2 == 0 else (
                    nc.scalar, nc.sync
                )
                qA.dma_start(wq_sb[:, k, :], wqT[k * P : (k + 1) * P, :])
                qB.dma_start(wk_sb[:, k, :], wkT[k * P : (k + 1) * P, :])
                # halves of each plane stream on both queues in parallel
                # so plane delivery keeps up with the projection matmuls
                qA.dma_start(
                    xT_sb[:, k, 0:1024], xT[k * P : (k + 1) * P, 0:1024]
                )
                qB.dma_start(
                    xT_sb[:, k, 1024:2048],
                    xT[k * P : (k + 1) * P, 1024:2048],
                )
            nc.gpsimd.dma_start(
                wv_sb[:], wvT.rearrange("(ko ki) j -> ki ko j", ki=P)
            )
            for c in range(3):
                nc.gpsimd.dma_start(
                    wo_sb[:, c, :], woS[c * P : (c + 1) * P, :]
                )

            # ones columns of V are static: set them once
            nc.vector.memset(v_sb[:, :, 192:193], 1.0)
            nc.vector.memset(v_sb[:, :, 385:386], 1.0)

            # multiplicative causal masks (0/1 bf16) for the 4 diagonal
            # sub-blocks, double width to cover both heads' fused at tile:
            # keep 1 iff q_local >= 128*d + k_local, else 0
            for d in range(4):
                nc.vector.memset(masks[:, d, :], 1.0)
                nc.gpsimd.affine_select(
                    out=masks[:, d, :],
                    in_=masks[:, d, :],
                    compare_op=mybir.AluOpType.is_ge,
                    fill=0.0,
                    base=-128 * d,
                    pattern=[[0, 2], [1, QB]],
                    channel_multiplier=-1,
                )

            # ---- Q/K projections (transposed outputs, 3 full planes) ----
            def wide_wave():
                # per xT plane: Q-c0's 4 groups + K-c0's first 2 groups
                # -> ~6 matmuls per plane arrival (cuts the DMA lead-in)
                pssQ = [
                    avps.tile([P, QB], fp32, tag=t, name=f"wwq_{t}")
                    for t in ("avA", "avB", "avC", "avD")
                ]
                pssK = [
                    scps.tile([P, QB], fp32, tag="sc", name=f"wwk_{i}")
                    for i in range(2)
                ]
                for k in range(KD):
                    for nt in range(NQ):
                        nc.tensor.matmul(
                            pssQ[nt],
                            lhsT=wq_sb[:, k, 0:P],
                            rhs=xT_sb[:, k, nt * QB : (nt + 1) * QB],
                            start=(k == 0), stop=(k == KD - 1),
                        )
                    for nt in range(2):
                        nc.tensor.matmul(
                            pssK[nt],
                            lhsT=wk_sb[:, k, 0:P],
                            rhs=xT_sb[:, k, nt * QB : (nt + 1) * QB],
                            start=(k == 0), stop=(k == KD - 1),
                        )
                for nt in range(NQ):
                    nc.scalar.copy(
                        qt_sb[:, 0, nt * QB : (nt + 1) * QB], pssQ[nt]
                    )
                for nt in range(2):
                    nc.scalar.copy(
                        kt_sb[:, 0, nt * QB : (nt + 1) * QB], pssK[nt]
                    )
                # K-c0's remaining 2 groups (planes all resident by now)
                for nt in (2, 3):
                    ps = avps.tile(
                        [P, QB], fp32, tag="av" + "ABCD"[nt], name=f"kc0{nt}"
                    )
                    for k in range(KD):
                        nc.tensor.matmul(
                            ps,
                            lhsT=wk_sb[:, k, 0:P],
                            rhs=xT_sb[:, k, nt * QB : (nt + 1) * QB],
                            start=(k == 0), stop=(k == KD - 1),
                        )
                    nc.scalar.copy(
                        kt_sb[:, 0, nt * QB : (nt + 1) * QB], ps
                    )

            def projections(first=False):
                if first:
                    wide_wave()
                for w_sb, o_sb in ((wq_sb, qt_sb), (wk_sb, kt_sb)):
                    for c in range(1 if first else 0, 3):
                        for nt in range(NQ):
                            ps = avps.tile(
                                [P, QB], fp32,
                                tag="av" + "ABCD"[nt], name=f"pj{c}{nt}",
                            )
                            for k in range(KD):
                                nc.tensor.matmul(
                                    ps,
                                    lhsT=w_sb[:, k, c * P : (c + 1) * P],
                                    rhs=xT_sb[:, k, nt * QB : (nt + 1) * QB],
                                    start=(k == 0),
                                    stop=(k == KD - 1),
                                )
                            nc.scalar.copy(
                                o_sb[:, c, nt * QB : (nt + 1) * QB], ps
                            )
                # ---- V projection (natural layout [h0 | h1]) ----
                for m in range(MS):
                    ps = avps.tile(
                        [P, QB], fp32, tag="av" + "ABCD"[m % 4], name=f"pv{m}"
                    )
                    for k in range(KD):
                        nc.tensor.matmul(
                            ps[:, 0:HD],
                            lhsT=xT_sb[:, k, m * P : (m + 1) * P],
                            rhs=wv_sb[:, k, :],
                            start=(k == 0),
                            stop=(k == KD - 1),
                        )
                    nc.scalar.copy(v_sb[:, m, 0:192], ps[:, 0:192])
                    nc.vector.tensor_copy(v_sb[:, m, 193:385], ps[:, 192:384])

            # ---- attention per q-block; out-proj deferred one block ----
            def out_proj(qj, mis=(0, 1, 2, 3), deep_psum=False):
                for mi in mis:
                    m = qj * 4 + mi
                    ost = ostp.tile([P, D], bf16, tag="ost")
                    for n in range(2):
                        # tail blocks rotate through the freed AV psum
                        # banks too, so psum reuse never waits on a copy
                        if deep_psum and (2 * mi + n) % 3:
                            pool, tag = avps, "av" + "ABCD"[(2 * mi + n) % 4]
                        else:
                            pool, tag = scps, "sc"
                        ps = pool.tile(
                            [P, QB], fp32, tag=tag, name=f"op{m}{n}"
                        )
                        for c in range(3):
                            nc.tensor.matmul(
                                ps[:, 0:384],
                                lhsT=pt_sb[:, c, m * P : (m + 1) * P],
                                rhs=wo_sb[:, c, n * 384 : (n + 1) * 384],
                                start=(c == 0),
                                stop=(c == 2),
                            )
                        # tail copies alternate DVE/ACT (ACT is idle by
                        # then) so the copy queue never gates psum reuse
                        (nc.scalar.copy if deep_psum and n == 1
                         else nc.vector.tensor_copy)(
                            ost[:, n * 384 : (n + 1) * 384], ps[:, 0:384]
                        )
                    # output DMAs stay off the sync queue (it carries the
                    # latency-critical transposes); the tail avoids the
                    # slow software-DGE gpsimd queue entirely
                    q = nc.scalar if deep_psum else (
                        [nc.scalar, nc.gpsimd][m % 2]
                    )
                    q.dma_start(out[m * P : (m + 1) * P, :], ost[:])

            def scores_part(qj, mask_eng=None):
                meng = mask_eng or nc.vector
                nk = 4 * qj + 4  # live key blocks (causal)
                # fused at tile: both heads side by side [.., h0 512 | h1 512]
                at2 = atp.tile(
                    [P, NK, 2 * QB], bf16, tag="at2", name=f"at2_{qj}"
                )
                for ki in range(nk):
                    ksl = slice(ki * KB, (ki + 1) * KB)
                    d = ki - 4 * qj
                    # diagonal blocks: only columns q >= 128*d are live
                    qo = 128 * d if d > 0 else 0
                    qsl = slice(qj * QB + qo, (qj + 1) * QB)
                    ps = scps.tile([P, 2 * QB], fp32, tag="sc")
                    ps0 = ps[:, qo:QB]
                    ps1 = ps[:, QB + qo : 2 * QB]
                    # full-plane matmuls (128 contraction rows)
                    nc.tensor.matmul(
                        ps0, lhsT=kt_sb[:, 0, ksl], rhs=qt_sb[:, 0, qsl],
                        start=True, stop=False,
                    )
                    nc.tensor.matmul(
                        ps1, lhsT=kt_sb[:, 1, ksl], rhs=qt_sb[:, 1, qsl],
                        start=True, stop=False,
                    )
                    # 64-row tails (packed plane 2)
                    nc.tensor.matmul(
                        ps0, lhsT=kt_sb[0:64, 2, ksl], rhs=qt_sb[0:64, 2, qsl],
                        start=False, stop=True,
                    )
                    nc.tensor.matmul(
                        ps1,
                        lhsT=kt_sb[64:128, 2, ksl],
                        rhs=qt_sb[64:128, 2, qsl],
                        start=False, stop=True,
                    )
                    # exp + post-exp causal zeroing; trimmed columns of
                    # diagonal blocks are never written nor read (the AV
                    # stage skips those (ki, q-sub-block) pairs entirely)
                    if qo == 0:
                        nc.scalar.activation(
                            at2[:, ki, :], ps, Exp, scale=SCALE
                        )
                        if d == 0:
                            meng.tensor_mul(
                                at2[:, ki, :], at2[:, ki, :], masks[:, d, :]
                            )
                    else:
                        for h0 in (0, QB):
                            nc.scalar.activation(
                                at2[:, ki, h0 + qo : h0 + QB],
                                ps[:, h0 + qo : h0 + QB],
                                Exp, scale=SCALE,
                            )
                            meng.tensor_mul(
                                at2[:, ki, h0 + qo : h0 + QB],
                                at2[:, ki, h0 + qo : h0 + QB],
                                masks[:, d, h0 + qo : h0 + QB],
                            )
                return at2

            def av_part(qj, at2, act_norm=False):
                # attention-stationary AV: per 128-query sub-block, per
                # head, accumulate at.T @ [V | ones] into a [q, 193] psum.
                # Column 192 is the softmax denominator (per-partition).
                for qs in range(4):
                    nkq = 4 * qj + qs + 1
                    # one 1-bank psum per head (an accumulation group may
                    # not share a psum zero region with another)
                    psA = avps.tile(
                        [P, QB], fp32, tag="av" + "AC"[qs % 2],
                        name=f"av{qj}{qs}h0",
                    )
                    psB = avps.tile(
                        [P, QB], fp32, tag="av" + "BD"[qs % 2],
                        name=f"av{qj}{qs}h1",
                    )
                    for ki in range(nkq):
                        nc.tensor.matmul(
                            psA[:, 0:193],
                            lhsT=at2[:, ki, qs * P : (qs + 1) * P],
                            rhs=v_sb[:, ki, 0:193],
                            start=(ki == 0), stop=(ki == nkq - 1),
                        )
                        nc.tensor.matmul(
                            psB[:, 0:193],
                            lhsT=at2[:, ki, QB + qs * P : QB + (qs + 1) * P],
                            rhs=v_sb[:, ki, 193:386],
                            start=(ki == 0), stop=(ki == nkq - 1),
                        )
                    rc = rcp.tile([P, 2], fp32, tag="rc")
                    nc.vector.reciprocal(rc[:, 0:1], psA[:, 192:193])
                    nc.vector.reciprocal(rc[:, 1:2], psB[:, 192:193])
                    nat = natp.tile([P, HD], bf16, tag="nat")
                    nc.vector.tensor_scalar_mul(
                        nat[:, 0:192], psA[:, 0:192], rc[:, 0:1]
                    )
                    # (an ACT-side copy-with-scale here regresses badly:
                    # activation-with-AP-scale stalls the exp pipeline)
                    nc.vector.tensor_scalar_mul(
                        nat[:, 192:384], psB[:, 0:192], rc[:, 1:2]
                    )
                    # async xbar transpose [q, d] -> [d-plane, q] into pt.
                    # All on the sync queue: a second transpose queue races
                    # (intermittent corruption) and a wider [128,3,256] out
                    # shape mis-addresses on HW, so exactly this shape.
                    nc.sync.dma_start_transpose(
                        pt_sb[
                            :, 0:3,
                            qj * QB + qs * P : qj * QB + (qs + 1) * P,
                        ],
                        nat[:],
                    )

            # big q-blocks first; every AV waits one-block-deferred so the
            # next block's scores cover its exp tail, and out-proj halves
            # fill remaining PE slack
            for _rep in range(reps):
                projections(first=(_rep == 0))
                a3 = scores_part(3)
                a2 = scores_part(2)
                av_part(3, a3)
                a1 = scores_part(1, mask_eng=nc.gpsimd)
                out_proj(3, (0, 1))
                av_part(2, a2)
                a0 = scores_part(0, mask_eng=nc.gpsimd)
                out_proj(3, (2, 3))
                out_proj(2, (0, 1))
                av_part(1, a1, act_norm=True)
                out_proj(2, (2, 3))
                out_proj(1, (0, 1))
                av_part(0, a0, act_norm=True)
                out_proj(0, deep_psum=True)
                out_proj(1, (2, 3), deep_psum=True)

    nc.compile()
    return nc


def _shard_inputs(x, Wq, Wk, Wv, Wo):
    bf = ml_dtypes.bfloat16
    in_maps = []
    for core in range(8):
        b, hp = core // 2, core % 2
        cols = slice(hp * HD, (hp + 1) * HD)
        in_maps.append(
            {
                "xT": np.ascontiguousarray(x[b].T).astype(bf),
                "wqT": np.ascontiguousarray(Wq[cols, :].T[:, PQ]).astype(bf),
                "wkT": np.ascontiguousarray(Wk[cols, :].T[:, PQ]).astype(bf),
                "wvT": np.ascontiguousarray(Wv[cols, :].T).astype(bf),
                "woS": np.ascontiguousarray(Wo[:, cols].T).astype(bf),
            }
        )
    return in_maps


def _run(inputs, trace=False, **kw):
    from concourse.bass_utils import run_bass_kernel_spmd

    if "nc" not in _CACHED:
        _CACHED["nc"] = build_nc()
    nc = _CACHED["nc"]
    in_maps = _shard_inputs(
        np.asarray(inputs["x"], np.float32),
        np.asarray(inputs["Wq"], np.float32),
        np.asarray(inputs["Wk"], np.float32),
        np.asarray(inputs["Wv"], np.float32),
        np.asarray(inputs["Wo"], np.float32),
    )
    res = run_bass_kernel_spmd(
        nc, in_maps, core_ids=list(range(8)), trace=trace, **kw
    )
    parts = [np.asarray(r["out"], np.float32) for r in res.results]
    full = np.empty((B, S, D), np.float32)
    for b in range(B):
        full[b] = parts[2 * b] + parts[2 * b + 1]
    return full, res


def kernel(**inputs) -> np.ndarray:
    full, _ = _run(inputs, trace=False)
    return full
